# revision 20
# baseline (speedup 1.0000x reference)
"""CLUB-NCE loss kernel for 8x Trainium2 NeuronCores (Bass/Tile).

Math (reference):
  hx = x @ W1x.T, hy = y @ W1y.T            [N, H]
  s[i,j]  = W2 . relu(hy[i] + hx[j] + b1) + b2
  T1[i,j] = softplus(s[i,j]); T0[i] = T1[i,i]
  lower = mean(T0) - (mean_i(logsumexp_j(T1[i,:])) - log N)
  upper = mean(T0) - mean(T1)

Sharding: y rows (i axis) split across 8 cores (64 rows each); x and MLP
params replicated. Each core computes its [64, 512] score block and emits
per-row partials (row sum of e^s, row sum of T1, diag e^s). Host combines.

Device design notes:
 - contraction dim k (=H=400) on partitions, tiled [128,128,128,16(+1)].
 - score row i is routed to PSUM partition i via a shifted one-hot
   stationary matrix: bsh[k] is [Pk, 128] with w2[k-chunk] at column 64,
   so lhsT = bsh[k][:, 64-ii : 96-ii] puts w2 in column ii.  All matmuls
   of a 32-row half accumulate into one [32, 512] PSUM bank; rows not
   owned by a matmul get exact +0.
 - the 16-deep k3 tail tiles of 4 consecutive rows are packed into one
   [64, 512] rhs and contracted by a single matmul using a banded
   stationary B3 (B3[p, 28 + p//16] = w2[384 + p%16], window slid by
   4 columns per quad): 13 matmuls per 4 rows instead of 16.
 - b1 is folded into the hy matmul as a virtual k=400 row.
 - PE warmup: dummy matmuls keep the tensor engine continuously busy
   from t~0.7us so it reaches full clock before real work, and filler
   dummies bridge the prologue->main handoff (any idle gap drops the
   PE clock for ~3us).
 - prologue is k-batch ordered (one batch per arriving input slab) so
   the PE never stalls mid-prologue.
 - two 32-row halves; half A's epilogue (Exp + softplus row sums on ACT,
   masked diag on DVE) hides under half B's matmuls.
 - per quad of rows: 12+1 relu tiles split DVE (4x mode) / ACT.
 - inputs arrive in 6 DMAs (one [*,1504] fp16 slab per k-tile + banded
   B3 + fp16 diag mask) to minimize serialized HWDGE occupancy.
 - host finishes: lse_i = log(N + rr_i), t0_i = log(1 + ed_i), means.
"""

import numpy as np

N = 512          # number of samples
D = 400          # feature dim
H = 400          # hidden dim
NCORES = 8
NL = N // NCORES  # 64 y-rows per core
NH = NL // 2      # 32 rows per half
KT = 4            # k tiles
KSZ = [128, 128, 128, 16]    # real k per tile (400 total)
KSZY = [128, 128, 128, 17]   # hy matmul k per tile (incl. bias row)
# consolidated input slabs, split in two pieces per k-tile so the hx
# matmuls can start before the hy-side data arrives:
#   A: x | w1x(+m3 dup)       B: w1y(+m3 dup) | yt | bsh
CX, CW1X = 0, 512
ATOT = 928
CW1Y, CYT, CBSH = 0, 416, 480
BTOT = 608
MSZ = [128, 128, 128, 32]    # H-tile partition sizes (m3 duplicated 16+16)
NWARM = 1         # PE warmup dummy matmuls (anchors the p-state ramp)
NFILL = 0         # PE filler dummies between prologue and main loop
DROWS = 384       # dummy matmul free size


def _build_program(b2val: float, enable_asserts: bool = False):
    import concourse.bacc as bacc
    import concourse.mybir as mybir
    import concourse.tile as tile

    fp16 = mybir.dt.float16
    f32 = mybir.dt.float32
    AF = mybir.ActivationFunctionType
    ALU = mybir.AluOpType

    nc = bacc.Bacc(
        "TRN2",
        target_bir_lowering=False,
        debug=False,
        enable_asserts=enable_asserts,
    )

    slabA = nc.dram_tensor("slabA", [401, ATOT], fp16, kind="ExternalInput")
    slabB = nc.dram_tensor("slabB", [401, BTOT], fp16, kind="ExternalInput")
    b3d = nc.dram_tensor("b3d", [128, 60], fp16, kind="ExternalInput")
    # x columns are rotated per core so the diag block sits at columns
    # [h*32, h*32+32) of half h; the mask is just a [32,32] identity
    maskd = nc.dram_tensor("maskd", [NH, NH], fp16, kind="ExternalInput")
    out_o = nc.dram_tensor("out_o", [NL, 4], f32, kind="ExternalOutput")

    with tile.TileContext(nc) as tc:
        with (
            tc.tile_pool(name="const", bufs=1) as cpool,
            tc.tile_pool(name="work", bufs=24) as wpool,
            tc.tile_pool(name="rq", bufs=4) as rqpool,
            tc.tile_pool(name="epi", bufs=2) as epool,
            tc.tile_pool(name="ppro", bufs=4, space="PSUM") as ppro,
            tc.tile_pool(name="phy", bufs=1, space="PSUM") as phy,
            tc.tile_pool(name="pmain", bufs=1, space="PSUM") as pmain,
            tc.tile_pool(name="pdum", bufs=1, space="PSUM") as pdum,
        ):
            # one table load (natural_log_exp_and_others: copy/relu/exp/ln)
            # hidden under the input DMAs instead of mid-epilogue
            nc.scalar.add_instruction(
                mybir.InstLoadActFuncSet(
                    name=nc.get_next_instruction_name(),
                    act_func_set_id=6,
                    engine=mybir.EngineType.Activation,
                    ins=[],
                    outs=[],
                )
            )

            # ---- input DMAs: A pieces first (feed hx), then B + B3 + mask ----
            sa_t, sb_t = [], []
            for k in range(KT):
                t = cpool.tile([KSZ[k], ATOT], fp16, name=f"slabA{k}")
                nc.sync.dma_start(
                    out=t, in_=slabA[k * 128 : k * 128 + KSZ[k], :]
                )
                sa_t.append(t)
            for k in range(KT):
                t = cpool.tile([KSZY[k], BTOT], fp16, name=f"slabB{k}")
                nc.sync.dma_start(
                    out=t, in_=slabB[k * 128 : k * 128 + KSZY[k], :]
                )
                sb_t.append(t)
            b3 = cpool.tile([128, 60], fp16, name="b3")
            nc.sync.dma_start(out=b3, in_=b3d[:, :])
            mask = cpool.tile([NH, NH], fp16, name="mask")
            nc.sync.dma_start(out=mask, in_=maskd[:, :])

            xt = [sa_t[k][:, CX : CX + N] for k in range(KT)]
            w1x = [sa_t[k][:, CW1X : CW1X + H + 16] for k in range(KT)]
            w1y = [sb_t[k][:, CW1Y : CW1Y + H + 16] for k in range(KT)]
            yt = [sb_t[k][:, CYT : CYT + NL] for k in range(KT)]
            bshl = [sb_t[k][: KSZ[k], CBSH : CBSH + 128] for k in range(KT)]

            # ---- PE warmup: keep the tensor engine busy from t~0 ----
            dumw = cpool.tile([128, 1], fp16, name="dumw")
            nc.vector.memset(dumw, 0.0)
            dumr = cpool.tile([128, DROWS], fp16, name="dumr")
            nc.vector.memset(dumr, 0.0)
            pd = pdum.tile([1, DROWS], f32, name="pd", tag="pd")

            b2t = cpool.tile([NH, 1], f32, name="b2t")
            nc.vector.memset(b2t, b2val)
            onet = cpool.tile([NH, 1], f32, name="onet")
            nc.vector.memset(onet, 1.0)
            out3 = cpool.tile([NL, 4], f32, name="out3")
            nc.vector.memset(out3, 0.0)

            def dummies(n):
                for _ in range(n):
                    nc.tensor.matmul(pd, lhsT=dumw, rhs=dumr,
                                     start=True, stop=True)

            dummies(NWARM)

            # ---- prologue, k-batch ordered: hy then hx per arriving slab ----
            pyall = phy.tile([128, 4 * NL], f32, name="pyall", tag="py")
            ph = [
                ppro.tile([MSZ[m], N], f32, name=f"ph{m}", tag="pp")
                for m in range(KT)
            ]
            # hx matmuls k-batched (one batch per arriving slab, no stalls;
            # the 4 ph banks are distinct so group interleaving is safe)
            for k in range(KT):
                for m in range(KT):
                    msl = slice(m * 128, m * 128 + MSZ[m])
                    nc.tensor.matmul(
                        ph[m], lhsT=w1x[k][:, msl], rhs=xt[k],
                        start=(k == 0), stop=(k == KT - 1),
                    )
            # hy blocks share one PSUM bank: groups must be sequential per
            # block (same-bank interleaved start/stop corrupts accumulation)
            for m in range(KT):
                msl = slice(m * 128, m * 128 + MSZ[m])
                for k in range(KT):
                    nc.tensor.matmul(
                        pyall[: MSZ[m], m * NL : (m + 1) * NL],
                        lhsT=w1y[k][:, msl], rhs=yt[k],
                        start=(k == 0), stop=(k == KT - 1),
                    )
            # hyb = hy + b1 (one f32 copy), then hx tiles to fp16
            hyball = cpool.tile([128, 4 * NL], f32, name="hyball")
            nc.vector.tensor_copy(out=hyball, in_=pyall)
            hx = []
            for m in range(KT):
                hxm = cpool.tile([MSZ[m], N], fp16, name=f"hx{m}")
                if m % 2 == 0:
                    nc.vector.tensor_copy(out=hxm, in_=ph[m])
                else:
                    nc.scalar.activation(
                        out=hxm, in_=ph[m], func=AF.Copy, bias=0.0, scale=1.0,
                    )
                hx.append(hxm)

            def hyb(m, i):  # per-partition scalar for H-tile m, row i
                return hyball[: MSZ[m], m * NL + i : m * NL + i + 1]

            dummies(NFILL)  # bridge prologue->main while copies drain

            # ---- main loop: two 32-row halves, quads of 4 rows ----
            def emit_quad(half, q):
                rq = rqpool.tile([128, N], fp16, name="rq", tag="rq")
                rks = []
                for a in range(4):
                    i = half * NH + 4 * q + a
                    for k in range(3):
                        r = wpool.tile([128, N], fp16, name=f"r{k}",
                                       tag=f"r{k}")
                        nc.vector.tensor_scalar(
                            out=r, in0=hx[k], scalar1=hyb(k, i), scalar2=0.0,
                            op0=ALU.add, op1=ALU.max,
                        )
                        rks.append(r)
                    if a == 0:
                        nc.vector.tensor_scalar(
                            out=rq[0:32, :], in0=hx[3],
                            scalar1=hyb(3, i), scalar2=0.0,
                            op0=ALU.add, op1=ALU.max,
                        )
                    else:
                        nc.scalar.activation(
                            out=rq[32 * a : 32 * (a + 1), :], in_=hx[3],
                            func=AF.Relu, bias=hyb(3, i), scale=1.0,
                        )
                for a in range(4):
                    ii = 4 * q + a
                    for k in range(3):
                        nc.tensor.matmul(
                            ps_h[half], lhsT=bshl[k][:, 64 - ii : 96 - ii],
                            rhs=rks[3 * a + k],
                            start=(q == 0 and a == 0 and k == 0), stop=False,
                        )
                nc.tensor.matmul(
                    ps_h[half], lhsT=b3[:, 28 - 4 * q : 60 - 4 * q], rhs=rq,
                    start=False, stop=(q == NH // 4 - 1),
                )

            def emit_epilogue(half):
                osl = slice(half * NH, (half + 1) * NH)
                e2 = epool.tile([NH, N], fp16, name="e2", tag="e2")
                t1s = epool.tile([NH, N], fp16, name="t1s", tag="t1s")
                tmp = epool.tile([NH, NH], fp16, name="tmp", tag="tmp")
                # E = exp(s + b2); rr = row sums of E
                nc.scalar.activation(
                    out=e2, in_=ps_h[half], func=AF.Exp, bias=b2t, scale=1.0,
                    accum_out=out3[osl, 0:1],
                )
                # T1 = log(1 + E); rs = row sums of T1
                nc.scalar.activation(
                    out=t1s, in_=e2, func=AF.Ln, bias=onet, scale=1.0,
                    accum_out=out3[osl, 1:2],
                )
                # ed = diag(E): rotated x puts the diag block at a fixed
                # 32-column window
                nc.vector.tensor_tensor(
                    out=tmp,
                    in0=e2[:, half * NH : (half + 1) * NH], in1=mask,
                    op=ALU.mult,
                )
                nc.vector.reduce_sum(
                    out=out3[osl, 2:3], in_=tmp, axis=mybir.AxisListType.X
                )
                nc.sync.dma_start(out=out_o[osl, :], in_=out3[osl, :])

            ps_h = [
                pmain.tile([NH, N], f32, name=f"ps{h}", tag=f"ps{h}")
                for h in range(2)
            ]
            for q in range(NH // 4):
                emit_quad(0, q)
            for q in range(NH // 4):
                emit_quad(1, q)
                if q == 0:
                    emit_epilogue(0)
            emit_epilogue(1)

    nc.compile()
    return nc


def _make_in_maps(x, y, W1, b1, W2):
    f16 = np.float16
    slabA = np.zeros((401, ATOT), f16)
    slabB = np.zeros((401, BTOT), f16)
    w1xT = W1[:, :D].T.astype(f16)       # [D(k), H(m)]
    w1yT = W1[:, D:].T.astype(f16)
    slabA[:D, CW1X : CW1X + H] = w1xT
    slabA[:D, CW1X + H : CW1X + H + 16] = w1xT[:, 384:400]   # m3 dup
    slabB[:D, CW1Y : CW1Y + H] = w1yT
    slabB[:D, CW1Y + H : CW1Y + H + 16] = w1yT[:, 384:400]
    slabB[400, CW1Y : CW1Y + H] = b1.astype(f16)
    slabB[400, CW1Y + H : CW1Y + H + 16] = b1[384:400].astype(f16)
    slabB[:H, CBSH + 64] = W2[0].astype(f16)
    b3p = np.zeros((128, 60), f16)
    p = np.arange(128)
    val = W2[0, 384 + (p % 32) % 16].astype(f16)
    b3p[p, 28 + p // 32] = np.where(p % 32 < 16, val, 0.0)

    maskp = np.eye(NH, dtype=f16)
    xT = x.T.astype(f16)
    in_maps = []
    for c in range(NCORES):
        sa = slabA.copy()
        # rotate x columns so core c's diag block lands at columns [0, 64)
        sa[:D, CX : CX + N] = np.roll(xT, -c * NL, axis=1)
        sb = slabB.copy()
        sb[:D, CYT : CYT + NL] = y[c * NL : (c + 1) * NL, :].T.astype(f16)
        sb[400, CYT : CYT + NL] = 1.0
        in_maps.append({"slabA": sa, "slabB": sb, "b3d": b3p, "maskd": maskp})
    return in_maps


def _combine(results):
    rr = np.concatenate([r["out_o"][:, 0].astype(np.float64) for r in results])
    rs = np.concatenate([r["out_o"][:, 1].astype(np.float64) for r in results])
    ed = np.concatenate([r["out_o"][:, 2].astype(np.float64) for r in results])
    lse = np.log(np.float64(N) + rr)
    t0 = np.log1p(ed)
    t0_mean = t0.mean()
    lower = t0_mean - (lse.mean() - np.log(np.float64(N)))
    upper = t0_mean - rs.mean() / N
    return np.float32(lower), np.float32(upper)


def kernel(x_samples, y_samples, W1, b1, W2, b2, _trace=False):
    from concourse.bass_utils import run_bass_kernel_spmd

    nc = _build_program(float(np.float32(b2[0])))
    in_maps = _make_in_maps(
        np.asarray(x_samples, np.float32),
        np.asarray(y_samples, np.float32),
        np.asarray(W1, np.float32),
        np.asarray(b1, np.float32),
        np.asarray(W2, np.float32),
    )
    res = run_bass_kernel_spmd(
        nc, in_maps, core_ids=list(range(NCORES)), trace=_trace
    )
    out = _combine(res.results)
    if _trace:
        return out, res
    return out


# revision 23
# speedup vs baseline: 1.0216x; 1.0216x over previous
"""CLUB-NCE loss kernel for 8x Trainium2 NeuronCores (Bass/Tile).

Math (reference):
  hx = x @ W1x.T, hy = y @ W1y.T            [N, H]
  s[i,j]  = W2 . relu(hy[i] + hx[j] + b1) + b2
  T1[i,j] = softplus(s[i,j]); T0[i] = T1[i,i]
  lower = mean(T0) - (mean_i(logsumexp_j(T1[i,:])) - log N)
  upper = mean(T0) - mean(T1)

Sharding: y rows (i axis) split across 8 cores (64 rows each); x and MLP
params replicated. Each core computes its [64, 512] score block and emits
per-row partials (row sum of e^s, row sum of T1, diag e^s). Host combines.

Device design notes:
 - contraction dim k (=H=400) on partitions, tiled [128,128,128,16(+1)].
 - score row i is routed to PSUM partition i via a shifted one-hot
   stationary matrix: bsh[k] is [Pk, 128] with w2[k-chunk] at column 64,
   so lhsT = bsh[k][:, 64-ii : 96-ii] puts w2 in column ii.  All matmuls
   of a 32-row half accumulate into one [32, 512] PSUM bank; rows not
   owned by a matmul get exact +0.
 - the 16-deep k3 tail tiles of 4 consecutive rows are packed into one
   [64, 512] rhs and contracted by a single matmul using a banded
   stationary B3 (B3[p, 28 + p//16] = w2[384 + p%16], window slid by
   4 columns per quad): 13 matmuls per 4 rows instead of 16.
 - b1 is folded into the hy matmul as a virtual k=400 row.
 - PE warmup: dummy matmuls keep the tensor engine continuously busy
   from t~0.7us so it reaches full clock before real work, and filler
   dummies bridge the prologue->main handoff (any idle gap drops the
   PE clock for ~3us).
 - prologue is k-batch ordered (one batch per arriving input slab) so
   the PE never stalls mid-prologue.
 - two 32-row halves; half A's epilogue (Exp + softplus row sums on ACT,
   masked diag on DVE) hides under half B's matmuls.
 - per quad of rows: 12+1 relu tiles split DVE (4x mode) / ACT.
 - inputs arrive in 6 DMAs (one [*,1504] fp16 slab per k-tile + banded
   B3 + fp16 diag mask) to minimize serialized HWDGE occupancy.
 - host finishes: lse_i = log(N + rr_i), t0_i = log(1 + ed_i), means.
"""

import numpy as np

N = 512          # number of samples
D = 400          # feature dim
H = 400          # hidden dim
NCORES = 8
NL = N // NCORES  # 64 y-rows per core
NH = NL // 2      # 32 rows per half
KT = 4            # k tiles
KSZ = [128, 128, 128, 16]    # real k per tile (400 total)
KSZY = [128, 128, 128, 17]   # hy matmul k per tile (incl. bias row)
# consolidated input slabs, split in two pieces per k-tile so the hx
# matmuls can start before the hy-side data arrives:
#   A: x | w1x m0-2 | w1x m3-oct (8 copies of the 16 tail columns)
#   B: w1y m0-2 | w1y3 parity blocks | yt | bsh
CX, CW1X = 0, 512
ATOT = 1024
CW1Y, CYT, CBSH = 0, 432, 496
BTOT = 624
MSZ = [128, 128, 128, 128]   # H-tile partition sizes (m3 oct-duplicated)
NWARM = 1         # PE warmup dummy matmuls (anchors the p-state ramp)
NFILL = 0         # PE filler dummies between prologue and main loop
DROWS = 384       # dummy matmul free size


def _build_program(b2val: float, enable_asserts: bool = False):
    import concourse.bacc as bacc
    import concourse.mybir as mybir
    import concourse.tile as tile

    fp16 = mybir.dt.float16
    f32 = mybir.dt.float32
    AF = mybir.ActivationFunctionType
    ALU = mybir.AluOpType

    nc = bacc.Bacc(
        "TRN2",
        target_bir_lowering=False,
        debug=False,
        enable_asserts=enable_asserts,
    )

    slabA = nc.dram_tensor("slabA", [401, ATOT], fp16, kind="ExternalInput")
    slabB = nc.dram_tensor("slabB", [401, BTOT], fp16, kind="ExternalInput")
    b3d = nc.dram_tensor("b3d", [128, 56], fp16, kind="ExternalInput")
    # x columns are rotated per core so the diag block sits at columns
    # [h*32, h*32+32) of half h; the mask is just a [32,32] identity
    maskd = nc.dram_tensor("maskd", [NH, NH], fp16, kind="ExternalInput")
    out_o = nc.dram_tensor("out_o", [NL, 4], f32, kind="ExternalOutput")

    with tile.TileContext(nc) as tc:
        with (
            tc.tile_pool(name="const", bufs=1) as cpool,
            tc.tile_pool(name="work", bufs=24) as wpool,
            tc.tile_pool(name="rq", bufs=4) as rqpool,
            tc.tile_pool(name="epi", bufs=2) as epool,
            tc.tile_pool(name="ppro", bufs=4, space="PSUM") as ppro,
            tc.tile_pool(name="phy", bufs=1, space="PSUM") as phy,
            tc.tile_pool(name="pmain", bufs=1, space="PSUM") as pmain,
            tc.tile_pool(name="pdum", bufs=1, space="PSUM") as pdum,
        ):
            # one table load (natural_log_exp_and_others: copy/relu/exp/ln)
            # hidden under the input DMAs instead of mid-epilogue
            nc.scalar.add_instruction(
                mybir.InstLoadActFuncSet(
                    name=nc.get_next_instruction_name(),
                    act_func_set_id=6,
                    engine=mybir.EngineType.Activation,
                    ins=[],
                    outs=[],
                )
            )

            # ---- input DMAs: A pieces first (feed hx), then B + B3 + mask ----
            sa_t, sb_t = [], []
            for k in range(KT):
                t = cpool.tile([KSZ[k], ATOT], fp16, name=f"slabA{k}")
                nc.sync.dma_start(
                    out=t, in_=slabA[k * 128 : k * 128 + KSZ[k], :]
                )
                sa_t.append(t)
            for k in range(KT):
                t = cpool.tile([KSZY[k], BTOT], fp16, name=f"slabB{k}")
                nc.sync.dma_start(
                    out=t, in_=slabB[k * 128 : k * 128 + KSZY[k], :]
                )
                sb_t.append(t)
            b3 = cpool.tile([128, 56], fp16, name="b3")
            nc.sync.dma_start(out=b3, in_=b3d[:, :])
            mask = cpool.tile([NH, NH], fp16, name="mask")
            nc.sync.dma_start(out=mask, in_=maskd[:, :])

            xt = [sa_t[k][:, CX : CX + N] for k in range(KT)]
            w1x = [sa_t[k][:, CW1X : CW1X + 512] for k in range(KT)]
            w1y = [sb_t[k][:, CW1Y : CW1Y + 384] for k in range(KT)]
            w1y3e = [sb_t[k][:, CW1Y + 384 : CW1Y + 416] for k in range(KT)]
            w1y3o = [sb_t[k][:, CW1Y + 400 : CW1Y + 432] for k in range(KT)]
            yt = [sb_t[k][:, CYT : CYT + NL] for k in range(KT)]
            bshl = [sb_t[k][: KSZ[k], CBSH : CBSH + 128] for k in range(KT)]

            # ---- PE warmup: keep the tensor engine busy from t~0 ----
            dumw = cpool.tile([128, 1], fp16, name="dumw")
            nc.vector.memset(dumw, 0.0)
            dumr = cpool.tile([128, DROWS], fp16, name="dumr")
            nc.vector.memset(dumr, 0.0)
            pd = pdum.tile([1, DROWS], f32, name="pd", tag="pd")

            b2t = cpool.tile([NH, 1], f32, name="b2t")
            nc.vector.memset(b2t, b2val)
            onet = cpool.tile([NH, 1], f32, name="onet")
            nc.vector.memset(onet, 1.0)
            out3 = cpool.tile([NL, 4], f32, name="out3")
            nc.vector.memset(out3, 0.0)

            def dummies(n):
                for _ in range(n):
                    nc.tensor.matmul(pd, lhsT=dumw, rhs=dumr,
                                     start=True, stop=True)

            dummies(NWARM)

            # ---- prologue, k-batch ordered: hy then hx per arriving slab ----
            pyall = phy.tile([128, 3 * NL], f32, name="pyall", tag="py")
            ph = [
                ppro.tile([MSZ[m], N], f32, name=f"ph{m}", tag="pp")
                for m in range(KT)
            ]
            # hx matmuls k-batched (one batch per arriving slab, no stalls;
            # the 4 ph banks are distinct so group interleaving is safe)
            for k in range(KT):
                for m in range(KT):
                    msl = (slice(m * 128, (m + 1) * 128) if m < 3
                           else slice(384, 512))
                    nc.tensor.matmul(
                        ph[m], lhsT=w1x[k][:, msl], rhs=xt[k],
                        start=(k == 0), stop=(k == KT - 1),
                    )
            # hy blocks share one PSUM bank: groups must be sequential per
            # block (same-bank interleaved start/stop corrupts accumulation)
            for m in range(3):
                msl = slice(m * 128, (m + 1) * 128)
                for k in range(KT):
                    nc.tensor.matmul(
                        pyall[:, m * NL : (m + 1) * NL],
                        lhsT=w1y[k][:, msl], rhs=yt[k],
                        start=(k == 0), stop=(k == KT - 1),
                    )
            # h3: per-partition-paired hy tail for the oct matmuls.
            # h3[32a+16p+m, t] = hy3[m, y-row 8t+2a+p] + b1[384+m]
            h3lo = ppro.tile([64, 8], f32, name="h3lo", tag="pp")
            h3hi = ppro.tile([64, 8], f32, name="h3hi", tag="pp")
            for a in range(4):
                dst = (h3lo if a < 2 else h3hi)[
                    32 * (a % 2) : 32 * (a % 2) + 32, :
                ]
                for par in range(2):
                    lh = w1y3e if par == 0 else w1y3o
                    for k in range(KT):
                        nc.tensor.matmul(
                            dst,
                            lhsT=lh[k],
                            rhs=yt[k][:, 2 * a + par : NL : 8],
                            start=(par == 0 and k == 0),
                            stop=(par == 1 and k == KT - 1),
                        )
            # hyb = hy + b1 (one f32 copy), then hx tiles to fp16
            hyball = cpool.tile([128, 3 * NL], f32, name="hyball")
            nc.vector.tensor_copy(out=hyball, in_=pyall)
            h3s = cpool.tile([128, 8], f32, name="h3s")
            nc.vector.tensor_copy(out=h3s[0:64, :], in_=h3lo)
            nc.vector.tensor_copy(out=h3s[64:128, :], in_=h3hi)
            hx = []
            for m in range(KT):
                hxm = cpool.tile([MSZ[m], N], fp16, name=f"hx{m}")
                if m % 2 == 0:
                    nc.vector.tensor_copy(out=hxm, in_=ph[m])
                else:
                    nc.scalar.activation(
                        out=hxm, in_=ph[m], func=AF.Copy, bias=0.0, scale=1.0,
                    )
                hx.append(hxm)

            def hyb(m, i):  # per-partition scalar for H-tile m, row i
                return hyball[: MSZ[m], m * NL + i : m * NL + i + 1]

            dummies(NFILL)  # bridge prologue->main while copies drain

            # ---- main loop: two 32-row halves, octs of 8 rows ----
            def emit_oct(half, o):
                rq = rqpool.tile([128, N], fp16, name="rq", tag="rq")
                # k3 tails for rows 8o..8o+7 packed as 4 paired 32-blocks
                for a in range(4):
                    bsl = slice(32 * a, 32 * (a + 1))
                    if a == 0:
                        nc.vector.tensor_scalar(
                            out=rq[bsl, :], in0=hx[3][bsl, :],
                            scalar1=h3s[bsl, 4 * half + o : 4 * half + o + 1],
                            scalar2=0.0, op0=ALU.add, op1=ALU.max,
                        )
                    else:
                        nc.scalar.activation(
                            out=rq[bsl, :], in_=hx[3][bsl, :],
                            func=AF.Relu,
                            bias=h3s[bsl, 4 * half + o : 4 * half + o + 1],
                            scale=1.0,
                        )
                rks = []
                for rr in range(8):
                    i = half * NH + 8 * o + rr
                    for k in range(3):
                        r = wpool.tile([128, N], fp16, name=f"r{k}",
                                       tag=f"r{k}")
                        nc.vector.tensor_scalar(
                            out=r, in0=hx[k], scalar1=hyb(k, i), scalar2=0.0,
                            op0=ALU.add, op1=ALU.max,
                        )
                        rks.append(r)
                for rr in range(8):
                    ii = 8 * o + rr
                    for k in range(3):
                        nc.tensor.matmul(
                            ps_h[half], lhsT=bshl[k][:, 64 - ii : 96 - ii],
                            rhs=rks[3 * rr + k],
                            start=(o == 0 and rr == 0 and k == 0), stop=False,
                        )
                nc.tensor.matmul(
                    ps_h[half], lhsT=b3[:, 24 - 8 * o : 56 - 8 * o], rhs=rq,
                    start=False, stop=(o == NH // 8 - 1),
                )

            def emit_epilogue(half):
                osl = slice(half * NH, (half + 1) * NH)
                e2 = epool.tile([NH, N], fp16, name="e2", tag="e2")
                t1s = epool.tile([NH, N], fp16, name="t1s", tag="t1s")
                tmp = epool.tile([NH, NH], fp16, name="tmp", tag="tmp")
                # E = exp(s + b2); rr = row sums of E
                nc.scalar.activation(
                    out=e2, in_=ps_h[half], func=AF.Exp, bias=b2t, scale=1.0,
                    accum_out=out3[osl, 0:1],
                )
                # T1 = log(1 + E); rs = row sums of T1
                nc.scalar.activation(
                    out=t1s, in_=e2, func=AF.Ln, bias=onet, scale=1.0,
                    accum_out=out3[osl, 1:2],
                )
                # ed = diag(E): rotated x puts the diag block at a fixed
                # 32-column window
                nc.vector.tensor_tensor(
                    out=tmp,
                    in0=e2[:, half * NH : (half + 1) * NH], in1=mask,
                    op=ALU.mult,
                )
                nc.vector.reduce_sum(
                    out=out3[osl, 2:3], in_=tmp, axis=mybir.AxisListType.X
                )
                nc.sync.dma_start(out=out_o[osl, :], in_=out3[osl, :])

            ps_h = [
                pmain.tile([NH, N], f32, name=f"ps{h}", tag=f"ps{h}")
                for h in range(2)
            ]
            for o in range(NH // 8):
                emit_oct(0, o)
            for o in range(NH // 8):
                emit_oct(1, o)
                if o == 0:
                    emit_epilogue(0)
            emit_epilogue(1)

    nc.compile()
    return nc


def _make_in_maps(x, y, W1, b1, W2):
    f16 = np.float16
    slabA = np.zeros((401, ATOT), f16)
    slabB = np.zeros((401, BTOT), f16)
    w1xT = W1[:, :D].T.astype(f16)       # [D(k), H(m)]
    w1yT = W1[:, D:].T.astype(f16)
    slabA[:D, CW1X : CW1X + 384] = w1xT[:, :384]
    slabA[:D, CW1X + 384 : CW1X + 512] = np.tile(w1xT[:, 384:400], (1, 8))
    slabB[:D, CW1Y : CW1Y + 384] = w1yT[:, :384]
    slabB[400, CW1Y : CW1Y + 384] = b1[:384].astype(f16)
    # parity blocks: [384:416) = [w1y3 | 0], [400:432) = [0 | w1y3]
    slabB[:D, CW1Y + 384 : CW1Y + 400] = w1yT[:, 384:400]
    slabB[400, CW1Y + 384 : CW1Y + 400] = b1[384:400].astype(f16)
    slabB[:D, CW1Y + 416 : CW1Y + 432] = w1yT[:, 384:400]
    slabB[400, CW1Y + 416 : CW1Y + 432] = b1[384:400].astype(f16)
    slabB[:H, CBSH + 64] = W2[0].astype(f16)
    b3p = np.zeros((128, 56), f16)
    p = np.arange(128)
    b3p[p, 24 + 2 * (p // 32) + (p % 32) // 16] = W2[0, 384 + (p % 16)].astype(f16)

    maskp = np.eye(NH, dtype=f16)
    xT = x.T.astype(f16)
    in_maps = []
    for c in range(NCORES):
        sa = slabA.copy()
        # rotate x columns so core c's diag block lands at columns [0, 64)
        sa[:D, CX : CX + N] = np.roll(xT, -c * NL, axis=1)
        sb = slabB.copy()
        sb[:D, CYT : CYT + NL] = y[c * NL : (c + 1) * NL, :].T.astype(f16)
        sb[400, CYT : CYT + NL] = 1.0
        in_maps.append({"slabA": sa, "slabB": sb, "b3d": b3p, "maskd": maskp})
    return in_maps


def _combine(results):
    rr = np.concatenate([r["out_o"][:, 0].astype(np.float64) for r in results])
    rs = np.concatenate([r["out_o"][:, 1].astype(np.float64) for r in results])
    ed = np.concatenate([r["out_o"][:, 2].astype(np.float64) for r in results])
    lse = np.log(np.float64(N) + rr)
    t0 = np.log1p(ed)
    t0_mean = t0.mean()
    lower = t0_mean - (lse.mean() - np.log(np.float64(N)))
    upper = t0_mean - rs.mean() / N
    return np.float32(lower), np.float32(upper)


def kernel(x_samples, y_samples, W1, b1, W2, b2, _trace=False):
    from concourse.bass_utils import run_bass_kernel_spmd

    nc = _build_program(float(np.float32(b2[0])))
    in_maps = _make_in_maps(
        np.asarray(x_samples, np.float32),
        np.asarray(y_samples, np.float32),
        np.asarray(W1, np.float32),
        np.asarray(b1, np.float32),
        np.asarray(W2, np.float32),
    )
    res = run_bass_kernel_spmd(
        nc, in_maps, core_ids=list(range(NCORES)), trace=_trace
    )
    out = _combine(res.results)
    if _trace:
        return out, res
    return out
